# revision 4
# baseline (speedup 1.0000x reference)
"""DiscriminativeLoss kernel v2 for 8 trn2 NeuronCores (Bass/Tile).

Core c handles image b = c//2, pixel half h = c%2 (NPIX = 524288 px/core).

Pass 1 (pixel-major, bf16): per-class counts (DVE is_equal+accum) and
segment sums (one-hot matmuls, pixels on the contraction axis, 16 px-cols
per matmul).
Pairwise AllReduce of the [10, 17] stats between the two half-image cores.
Pass 2 (block-major, fp8): per chunk [80, 512] PSUM built by a mask matmul
(-B'*(la-ka)^2 -B'*(lb-kb)^2 from 4 aux label rows, e5m2 coeffs) plus ONE
fp8 DoubleRow matmul fusing -2*e.c (tile 0) and sum e^2 (tile 1); then
u = relu(psum + q_k - B'(ka^2+kb^2) - dvar^2) on DVE (7/8 of chunks) or
ACT (1/8) with per-lane accumulation, and y = sqrt(u + dvar^2) on ACT.
Wrong-class lanes land exactly at 0 through the whole chain.
Host: dtype/layout prep, final ~500-flop scalar assembly.
"""

import os
import sys

import numpy as np

sys.path.insert(0, "/opt/trn_rl_repo")
os.environ.setdefault("MYCRO_LOCAL_CACHE", "1")

import ml_dtypes  # noqa: E402

BF16 = ml_dtypes.bfloat16
FP8 = ml_dtypes.float8_e4m3
FP8E5 = ml_dtypes.float8_e5m2

# problem constants (hardcoded per harness contract)
B, E, H, W = 4, 16, 1024, 1024
NIMG = H * W
NCORES = 8
NPIX = NIMG // 2            # pixels per core
K = 10
KJ = 80                     # lane layout 8k+j
DELTA_VAR = 0.5
DELTA_DST = 1.5
A_W, B_W, R_W = 1.0, 1.0, 0.001
BIGM = 128.0                # wrong-class mask magnitude

F1 = 256                    # pass-1 pixel-cols per chunk
E2 = E + 1                  # emb channels + ones column (counts)
RUN = NPIX // 128           # 4096 pixel-cols per partition
NCH1 = RUN // F1            # 16
F2 = 512                    # pass-2 pixels per j-block per chunk
NCH2 = NPIX // (8 * F2)     # 128
NCH3 = (NCH2 + 2) // 3      # mask-row chunk columns (3-phase partition pack)

_cache = {}


def _consts():
    """Host-side constant arrays shared by all cores."""
    ka = np.arange(K) & 3
    kb = np.arange(K) >> 2
    # coefh: [128, 80] e5m2; rows 32*m + r*8 + j, col 8k+j' (nonzero j'==j)
    coef = np.zeros((4, 8, KJ), dtype=np.float32)   # [r, j, lane]
    for k in range(K):
        for j in range(8):
            coef[0, j, 8 * k + j] = 2.0 * BIGM * ka[k]
            coef[1, j, 8 * k + j] = -BIGM
            coef[2, j, 8 * k + j] = 2.0 * BIGM * kb[k]
            coef[3, j, 8 * k + j] = -BIGM
    coefh = np.tile(coef.reshape(32, KJ), (4, 1)).astype(FP8E5)
    # s2h: [128, 80] fp8: s2h[16j+e, 8k+j'] = (j' == j)
    s2h = np.zeros((128, KJ), dtype=np.float32)
    for j in range(8):
        for e in range(E):
            for k in range(K):
                s2h[16 * j + e, 8 * k + j] = 1.0
    # kpat: [128, K*F1] bf16: kpat[p, k*F1 + f] = k
    kpat = np.zeros((128, K * F1), dtype=np.float32)
    for k in range(K):
        kpat[:, k * F1:(k + 1) * F1] = float(k)
    # qsel: [10, 80]: qsel[k, 8k+j] = 1
    qsel = np.zeros((K, KJ), dtype=np.float32)
    for k in range(K):
        qsel[k, 8 * k:8 * k + 8] = 1.0
    # bkh: [80, 1] = -B'(ka^2+kb^2) - dvar^2
    bkh = np.zeros((KJ, 1), dtype=np.float32)
    for k in range(K):
        bkh[8 * k:8 * k + 8, 0] = (-BIGM * (ka[k] ** 2 + kb[k] ** 2)
                                   - DELTA_VAR * DELTA_VAR)
    # jcol: [80, 10]: jcol[8k+j, k] = 1
    jcol = np.zeros((KJ, K), dtype=np.float32)
    for k in range(K):
        jcol[8 * k:8 * k + 8, k] = 1.0
    return {
        "coefh": coefh,
        "s2h": s2h.astype(FP8),
        "kpat": kpat.astype(BF16),
        "qsel": qsel,
        "bkh": bkh,
        "jcol": jcol,
        "id10": np.eye(K, dtype=np.float32),
    }


def build_module():
    import concourse.mybir as mybir
    import concourse.tile as tile
    from concourse import bacc

    f32 = mybir.dt.float32
    bf16 = mybir.dt.bfloat16
    fp8 = mybir.dt.float8e4
    fp8e5 = mybir.dt.float8e5
    Alu = mybir.AluOpType
    Act = mybir.ActivationFunctionType
    DR = mybir.MatmulPerfMode.DoubleRow

    debug = os.environ.get("KV2_DEBUG", "0") == "1"
    nc = bacc.Bacc("TRN2", target_bir_lowering=False, debug=False,
                   num_devices=NCORES)

    # inputs
    embp_d = nc.dram_tensor("embp", [128, RUN * E2], fp8,
                            kind="ExternalInput").ap()
    labp_d = nc.dram_tensor("labp", [128, RUN], bf16,
                            kind="ExternalInput").ap()
    embb_d = nc.dram_tensor("embb", [128, NCH2 * 2 * F2], fp8,
                            kind="ExternalInput").ap()
    labb4_d = nc.dram_tensor("labb4", [128, NCH3 * F2], fp8,
                             kind="ExternalInput").ap()
    coefh_d = nc.dram_tensor("coefh", [128, KJ], fp8e5,
                             kind="ExternalInput").ap()
    s2h_d = nc.dram_tensor("s2h", [128, KJ], fp8, kind="ExternalInput").ap()
    kpat_d = nc.dram_tensor("kpat", [128, K * F1], bf16,
                            kind="ExternalInput").ap()
    qsel_d = nc.dram_tensor("qsel", [K, KJ], f32, kind="ExternalInput").ap()
    bkh_d = nc.dram_tensor("bkh", [KJ, 1], f32, kind="ExternalInput").ap()
    jcol_d = nc.dram_tensor("jcol", [KJ, K], f32, kind="ExternalInput").ap()
    id10_d = nc.dram_tensor("id10", [K, K], f32, kind="ExternalInput").ap()

    # outputs
    stats_d = nc.dram_tensor("stats", [K, 17], f32, kind="ExternalOutput").ap()
    hpart_d = nc.dram_tensor("hpart", [1, K], f32, kind="ExternalOutput").ap()
    if debug:
        dbg_d = nc.dram_tensor("dbg", [KJ, F2], f32,
                               kind="ExternalOutput").ap()
        dbgu_d = nc.dram_tensor("dbgu", [KJ, NCH2], f32,
                                kind="ExternalOutput").ap()
        dbgy_d = nc.dram_tensor("dbgy", [KJ, NCH2], f32,
                                kind="ExternalOutput").ap()

    with tile.TileContext(nc) as tc:
        with (
            tc.tile_pool(name="consts", bufs=1) as cp,
            tc.tile_pool(name="p1", bufs=3) as p1,
            tc.tile_pool(name="p2", bufs=3) as p2,
            tc.tile_pool(name="ps2", bufs=3, space="PSUM") as psp,
            tc.tile_pool(name="ps1", bufs=1, space="PSUM") as ps1,
            tc.tile_pool(name="dram", bufs=1, space="DRAM") as dp,
        ):
            # ---- small consts (SP queue, parallel to bulk stream) ----
            coef_t = cp.tile([128, KJ], fp8e5)
            nc.sync.dma_start(coef_t[:], coefh_d[:])
            kpat_t = cp.tile([128, K * F1], bf16)
            nc.sync.dma_start(kpat_t[:], kpat_d[:])
            qsel_t = cp.tile([K, KJ], f32)
            nc.sync.dma_start(qsel_t[:], qsel_d[:])
            bkh_t = cp.tile([KJ, 1], f32)
            nc.sync.dma_start(bkh_t[:], bkh_d[:])
            jcol_t = cp.tile([KJ, K], f32)
            nc.sync.dma_start(jcol_t[:], jcol_d[:])
            id10_t = cp.tile([K, K], f32)
            nc.sync.dma_start(id10_t[:], id10_d[:])

            # ---- AR-independent pass-2 setup (overlaps pass-1) ----
            s12_t = cp.tile([128, 2, KJ], fp8)
            nc.vector.memset(s12_t[:], 0.0)
            nc.gpsimd.dma_start(s12_t[:, 1, :], s2h_d[:])
            dv2 = cp.tile([KJ, 1], f32)
            nc.vector.memset(dv2[:], DELTA_VAR * DELTA_VAR)
            zer_t = cp.tile([KJ, 2 * F2], bf16)
            nc.vector.memset(zer_t[:], 0.0)

            # ---- pass-1 stream (gpsimd queue, highest priority) ----

            # segment sums (+counts via ones column) via one-hot matmuls
            sums_ps = ps1.tile([K, E2], f32, tag="ps_a")
            embp_r = embp_d.rearrange("p (c x) -> c p x", c=NCH1)
            labp_r = labp_d.rearrange("p (c f) -> c p f", c=NCH1)
            for c in range(NCH1):
                labc = p1.tile([128, F1], bf16, tag="labc")
                nc.gpsimd.dma_start(labc[:], labp_r[c])
                embc = p1.tile([128, F1 * E2], fp8, tag="embc")
                nc.gpsimd.dma_start(embc[:], embp_r[c])
                ohp = p1.tile([128, K * F1], bf16, tag="ohp")
                nc.vector.tensor_tensor(
                    out=ohp[:].rearrange("p (k f) -> p k f", f=F1),
                    in0=labc[:].unsqueeze(1).to_broadcast([128, K, F1]),
                    in1=kpat_t[:].rearrange("p (k f) -> p k f", f=F1),
                    op=Alu.is_equal)
                ohp_v = ohp[:].rearrange("p (k f) -> p f k", f=F1)
                emb_v = embc[:].rearrange("p (f e) -> p f e", e=E2)
                for f in range(F1):
                    nc.tensor.matmul(
                        sums_ps[:], lhsT=ohp_v[:, f, :], rhs=emb_v[:, f, :],
                        start=(c == 0 and f == 0),
                        stop=(c == NCH1 - 1 and f == F1 - 1))


            # ---- pass-2 bulk prefetch, gated behind pass-1 end so the
            # embp stream never starves (dummy write -> WAW dep on the DMA)
            embb_t = cp.tile([128, NCH2, 2, F2], fp8)
            labb4_t = cp.tile([128, NCH3, F2], fp8)
            labb4_r = labb4_d.rearrange("p (c f) -> p c f", f=F2)
            embb_r = embb_d.rearrange("p (c t f) -> p c t f", c=NCH2, t=2)
            for i in range(4):
                nc.scalar.activation(
                    labb4_t[0:1, 11 * i:11 * i + 1, 0:1],
                    sums_ps[0:1, 0:1], Act.Copy)
            for i in range(64):
                nc.scalar.activation(embb_t[0:1, 2 * i:2 * i + 1, 0:1, 0:1],
                                     sums_ps[0:1, 0:1], Act.Copy)
            for i in range(4):
                hi = min(11 * (i + 1), NCH3)
                nc.sync.dma_start(labb4_t[:, 11 * i:hi],
                                  labb4_r[:, 11 * i:hi])
            for i in range(64):
                nc.sync.dma_start(embb_t[:, 2 * i:2 * (i + 1)],
                                  embb_r[:, 2 * i:2 * (i + 1)])

            # ---- pairwise AllReduce with the half-image partner ----
            stats_blk = cp.tile([K, 17], f32)
            nc.scalar.copy(stats_blk[:], sums_ps[:])
            cc_in = dp.tile([K, 17], f32)
            cc_out = dp.tile([K, 17], f32)
            nc.gpsimd.dma_start(cc_in[:], stats_blk[:])
            nc.gpsimd.collective_compute(
                "AllReduce", mybir.AluOpType.add,
                replica_groups=[[0, 1], [2, 3], [4, 5], [6, 7]],
                ins=[cc_in[:].opt()], outs=[cc_out[:].opt()])
            stats_all = cp.tile([K, 17], f32)
            nc.gpsimd.dma_start(stats_all[:], cc_out[:])
            nc.gpsimd.dma_start(stats_d[:], cc_out[:])

            # ---- centers & pass-2 stationaries ----
            cnt_safe = cp.tile([K, 1], f32)
            nc.vector.tensor_scalar(out=cnt_safe[:], in0=stats_all[:, E:E2],
                                    scalar1=1.0, scalar2=None, op0=Alu.max)
            rec = cp.tile([K, 1], f32)
            nc.vector.reciprocal(rec[:], cnt_safe[:])
            cmat = cp.tile([K, E], f32)
            nc.vector.tensor_scalar(out=cmat[:], in0=stats_all[:, 0:E],
                                    scalar1=rec[:, 0:1], scalar2=None,
                                    op0=Alu.mult)
            csq = cp.tile([K, E], f32)
            nc.vector.tensor_tensor(csq[:], cmat[:], cmat[:], op=Alu.mult)
            qv = cp.tile([K, 1], f32)
            nc.vector.tensor_reduce(qv[:], csq[:], mybir.AxisListType.X,
                                    Alu.add)

            ct_ps = ps1.tile([E, K], f32, tag="ps_b")
            nc.tensor.matmul(ct_ps[:], lhsT=cmat[:], rhs=id10_t[:],
                             start=True, stop=True)
            ctbm = cp.tile([E, K], fp8)
            nc.scalar.activation(ctbm[:], ct_ps[:], Act.Copy, bias=0.0,
                                 scale=-2.0)
            s1v = s12_t[:, 0, :].rearrange("p (k j) -> p j k", j=8)
            for j in range(8):
                if j % 2 == 0:
                    nc.vector.tensor_scalar(
                        out=s1v[16 * j:16 * (j + 1), j, :], in0=ctbm[:],
                        scalar1=0.0, scalar2=None, op0=Alu.add)
                else:
                    nc.gpsimd.dma_start(s1v[16 * j:16 * (j + 1), j, :],
                                        ctbm[:])

            qb_ps = ps1.tile([KJ, 1], f32, tag="ps_b")
            nc.tensor.matmul(qb_ps[:], lhsT=qsel_t[:], rhs=qv[:],
                             start=True, stop=True)
            qb2 = cp.tile([KJ, 1], f32)
            nc.scalar.activation(qb2[:], qb_ps[:], Act.Identity,
                                 bias=bkh_t[:, 0:1], scale=1.0)
            # ---- pass 2 ----
            NPAIR = NCH2 // 2
            uaccV = cp.tile([KJ, NPAIR], f32)
            uaccA = cp.tile([KJ, NPAIR // 13 + 1], f32)
            yacc = cp.tile([KJ, NPAIR // 2], f32)
            nc.vector.memset(uaccV[:], 0.0)
            nc.vector.memset(uaccA[:], 0.0)
            tr_t = cp.tile([KJ, 4 * F2], bf16)
            for c2 in range(NPAIR):
                ps2 = psp.tile([KJ, 2, F2], f32, tag="ps2")
                for h in range(2):
                    c = 2 * c2 + h
                    m = c % 3
                    nc.tensor.matmul(
                        ps2[:, h, :], lhsT=coef_t[32 * m:32 * (m + 1), :],
                        rhs=labb4_t[32 * m:32 * (m + 1), c // 3, :],
                        start=True, stop=False)
                    nc.tensor.matmul(
                        ps2[:, h, :], lhsT=s12_t[:], rhs=embb_t[:, c],
                        perf_mode=DR, start=False, stop=True)
                if c2 % 2 == 0:
                    u4 = p2.tile([KJ, 4 * F2], bf16, tag="u4")
                usl = u4[:, (c2 % 2) * 2 * F2:(c2 % 2 + 1) * 2 * F2]
                if c2 % 13 == 0:
                    nc.scalar.activation(
                        usl, ps2[:], Act.Relu, bias=qb2[:, 0:1], scale=1.0,
                        accum_out=uaccA[:, c2 // 13:c2 // 13 + 1])
                else:
                    nc.vector.scalar_tensor_tensor(
                        out=usl, in0=ps2[:], scalar=qb2[:, 0:1],
                        in1=zer_t[:], op0=Alu.add, op1=Alu.max,
                        accum_out=uaccV[:, c2:c2 + 1])
                if c2 % 2 == 1:
                    nc.scalar.activation(
                        tr_t[:], u4[:], Act.Sqrt, bias=dv2[:, 0:1], scale=1.0,
                        accum_out=yacc[:, c2 // 2:c2 // 2 + 1])
                if debug and c == 1:
                    nc.sync.dma_start(dbg_d[:], u_t[:])

            # ---- H assembly: hp = sum(u) - 2d*sum(y) + 2d^2*npp ----
            if debug:
                nc.sync.dma_start(dbgu_d[:], uaccV[:])
                nc.sync.dma_start(dbgy_d[:], yacc[:])
            u1a = cp.tile([KJ, 1], f32)
            u1b = cp.tile([KJ, 1], f32)
            y1 = cp.tile([KJ, 1], f32)
            nc.vector.tensor_reduce(u1a[:], uaccV[:], mybir.AxisListType.X,
                                    Alu.add)
            nc.vector.tensor_reduce(u1b[:], uaccA[:], mybir.AxisListType.X,
                                    Alu.add)
            nc.vector.tensor_reduce(y1[:], yacc[:], mybir.AxisListType.X,
                                    Alu.add)
            u1 = cp.tile([KJ, 1], f32)
            nc.vector.tensor_tensor(u1[:], u1a[:], u1b[:], op=Alu.add)
            hp = cp.tile([KJ, 1], f32)
            nc.vector.scalar_tensor_tensor(
                out=hp[:], in0=y1[:], scalar=-2.0 * DELTA_VAR, in1=u1[:],
                op0=Alu.mult, op1=Alu.add)
            npp = float(F2 * NCH2)
            hp2 = cp.tile([KJ, 1], f32)
            nc.vector.tensor_scalar(
                out=hp2[:], in0=hp[:],
                scalar1=2.0 * DELTA_VAR * DELTA_VAR * npp,
                scalar2=None, op0=Alu.add)
            h_ps = ps1.tile([1, K], f32, tag="ps_a")
            nc.tensor.matmul(h_ps[:], lhsT=hp2[:], rhs=jcol_t[:],
                             start=True, stop=True)
            h_sb = cp.tile([1, K], f32)
            nc.scalar.copy(h_sb[:], h_ps[:])
            nc.sync.dma_start(hpart_d[:], h_sb[:])

    nc.compile()
    return nc


def _prep_core(esh, lab):
    """Per-core host buffers. esh: [E, NPIX] f32, lab: [NPIX] int."""
    out = {}
    # pixel-major [p][f][e] bf16; pixel = p*RUN + f
    pm = np.ones((128, RUN, E2), dtype=FP8)
    pm[:, :, :E] = esh.reshape(E, 128, RUN).transpose(1, 2, 0).astype(FP8)
    out["embp"] = pm.reshape(128, RUN * E2)
    out["labp"] = np.ascontiguousarray(
        lab.reshape(128, RUN).astype(np.float32)).astype(BF16)
    # block-major [16j+e][c][t][f]; pixel = c*4096 + j*512 + f
    v = esh.reshape(E, NCH2, 8, F2).transpose(2, 0, 1, 3)  # j e c f
    arr = np.empty((8, E, NCH2, 2, F2), dtype=FP8)
    arr[:, :, :, 0, :] = v.astype(FP8)
    arr[:, :, :, 1, :] = (v * v).astype(FP8)
    out["embb"] = arr.reshape(128, NCH2 * 2 * F2)
    # aux label rows [32*(c%3) + r*8 + j][c//3][f]
    labj = lab.reshape(NCH2, 8, F2).astype(np.int32)
    la, lb = labj & 3, labj >> 2
    a4 = np.stack([la, la * la, lb, lb * lb]).astype(np.float32)  # r c j f
    l4 = np.zeros((128, NCH3, F2), dtype=np.float32)
    for m in range(3):
        cs = np.arange(m, NCH2, 3)
        blk = a4[:, cs].transpose(0, 2, 1, 3).reshape(32, len(cs), F2)
        l4[32 * m:32 * m + 32, :len(cs)] = blk
    out["labb4"] = np.ascontiguousarray(l4.astype(FP8)).reshape(
        128, NCH3 * F2)
    return out


def prepare(embedding, ins_label):
    key = "mod"
    if key not in _cache:
        _cache[key] = build_module()
    nc = _cache[key]

    consts = _consts()
    emb_r = np.asarray(embedding, dtype=np.float32).reshape(B, E, NIMG)
    lab_r = np.asarray(ins_label).reshape(B, NIMG)

    in_maps = []
    for c in range(NCORES):
        b, h = c // 2, c % 2
        sl = slice(h * NPIX, (h + 1) * NPIX)
        m = dict(consts)
        m.update(_prep_core(np.ascontiguousarray(emb_r[b, :, sl]),
                            lab_r[b, sl]))
        in_maps.append(m)
    return nc, in_maps


def _host_finalize(stats, hsum):
    """stats: [B, 10, 17]; hsum: [B, 10] summed hinge partials."""
    lv_l, ld_l, lr_l, valid_l = [], [], [], []
    ids = np.arange(K)
    for b in range(B):
        counts = stats[b, :, 16].astype(np.float64)
        sums = stats[b, :, 0:16].astype(np.float64)
        present = (counts > 0) & (ids > 0)
        presf = present.astype(np.float64)
        safe = np.where(counts > 0, counts, 1.0)
        centers = sums / safe[:, None]
        per_inst = hsum[b].astype(np.float64) / safe
        n_inst = presf.sum()
        lv = float((per_inst * presf).sum() / max(n_inst, 1.0))
        cdiff = centers[:, None, :] - centers[None, :, :]
        csq = (cdiff * cdiff).sum(-1)
        pm = present[:, None] & present[None, :] & (ids[:, None] < ids[None, :])
        cdist = np.sqrt(np.where(pm, csq, 1.0))
        ph = np.square(np.maximum(2.0 * DELTA_DST - cdist, 0.0)) * pm
        n_pairs = pm.sum()
        ld = float(ph.sum() / max(n_pairs, 1.0))
        cn = np.sqrt(np.where(present, (centers * centers).sum(-1), 1.0))
        lr = float((cn * presf).sum() / max(n_inst, 1.0))
        valid = 1.0 if n_inst > 0 else 0.0
        lv_l.append(lv * valid)
        ld_l.append(ld * valid)
        lr_l.append(lr * valid)
        valid_l.append(valid)
    vb = max(sum(valid_l), 1.0)
    loss_var = sum(lv_l) / vb
    loss_dst = sum(ld_l) / vb
    loss_reg = sum(lr_l) / vb
    total = A_W * loss_var + B_W * loss_dst + R_W * loss_reg
    return (
        np.float32(total),
        np.float32(loss_var),
        np.float32(loss_dst),
        np.float32(loss_reg),
    )


def kernel(embedding, ins_label):
    from concourse.bass_utils import run_bass_kernel_spmd

    nc, in_maps = prepare(embedding, ins_label)
    res = run_bass_kernel_spmd(nc, in_maps, core_ids=list(range(NCORES)))
    stats = np.stack([res.results[2 * b]["stats"] for b in range(B)])
    hsum = np.zeros((B, K), dtype=np.float64)
    for c in range(NCORES):
        hsum[c // 2] += res.results[c]["hpart"].astype(np.float64).reshape(K)
    return _host_finalize(stats.astype(np.float64), hsum)


if __name__ == "__main__":
    build_module()
    print("build ok")
